# revision 10
# baseline (speedup 1.0000x reference)
"""SE (squeeze-excite) block for x[32,64,256,256] f32 on 8 TRN2 NeuronCores.

Data-parallel over batch: 4 batches per core, SE weights replicated.
Per core: x viewed as [256 rows = (4b x 64c), 65536 spatial].
  Pass 1: stream 16 tiles of [128, 8192], reduce_sum -> row sums.
  MLP:    two tiny PE matmuls (64->4 relu, 4->64 sigmoid), biases per
          partition on the scalar engine; 1/65536 mean-scale folded into
          the relu activation's scale.
  Pass 2: re-stream the 16 tiles, multiply by per-row sigmoid scale, store.
"""

import numpy as np

import concourse.bacc as bacc
import concourse.bass as bass
import concourse.mybir as mybir
from concourse import tile
from concourse.bass_utils import run_bass_kernel_spmd

N_CORES = 8
B, C, H, W = 32, 64, 256, 256
C_MID = 4
B_LOC = B // N_CORES            # 4 batches per core
ROWS = B_LOC * C                # 256 (b,c) rows per core
SPATIAL = H * W                 # 65536
NG = ROWS // 128                # 2 partition groups
NB_PER_G = 128 // C             # 2 batches per partition group
T = 8192                        # spatial chunk (32KB/partition, 4MiB/DMA)
NS = SPATIAL // T               # 8 chunks
F32 = mybir.dt.float32

TRACE = False
LAST_RESULT = None

_NC = None


def _build():
    global _NC
    if _NC is not None:
        return _NC

    nc = bacc.Bacc("TRN2", debug=False)

    x = nc.dram_tensor("x", [ROWS, SPATIAL], F32, kind="ExternalInput")
    wd = nc.dram_tensor("w_down", [C_MID, C], F32, kind="ExternalInput")
    bd = nc.dram_tensor("b_down", [C_MID], F32, kind="ExternalInput")
    wu = nc.dram_tensor("w_up", [C, C_MID], F32, kind="ExternalInput")
    bu = nc.dram_tensor("b_up", [C], F32, kind="ExternalInput")
    y = nc.dram_tensor("y", [ROWS, SPATIAL], F32, kind="ExternalOutput")

    x_t = x.ap().rearrange("(g p) (s t) -> g p s t", p=128, t=T)
    y_t = y.ap().rearrange("(g p) (s t) -> g p s t", p=128, t=T)

    with tile.TileContext(nc) as tc:
        with (
            tc.tile_pool(name="const", bufs=1) as cpool,
            tc.tile_pool(name="io", bufs=4) as io_pool,
            tc.tile_pool(name="stats", bufs=1) as spool,
            tc.tile_pool(name="psum", bufs=1, space=bass.MemorySpace.PSUM) as ppool,
        ):
            # --- tiny replicated weights -> SBUF, transposed for PE ---
            wdT = cpool.tile([C, C_MID], F32)           # [c, m] = w_down^T
            nc.sync.dma_start(wdT[:], wd.ap().rearrange("m c -> c m"))
            wuT = cpool.tile([C_MID, C], F32)           # [m, c] = w_up^T
            nc.sync.dma_start(wuT[:], wu.ap().rearrange("c m -> m c"))
            bdT = cpool.tile([C_MID, 1], F32)           # per-partition (m) bias
            nc.sync.dma_start(bdT[:], bd.ap().unsqueeze(1))
            buT = cpool.tile([C, 1], F32)               # per-partition (c) bias
            nc.sync.dma_start(buT[:], bu.ap().unsqueeze(1))

            # --- pass 1: row sums over spatial ---
            sums = spool.tile([128, NG, NS], F32)
            for g in range(NG):
                for s in range(NS):
                    tin = io_pool.tile([128, T], F32, tag="io")
                    nc.gpsimd.dma_start(tin[:], x_t[g, :, s, :])
                    nc.vector.reduce_sum(sums[:, g, s:s + 1], tin[:],
                                         axis=mybir.AxisListType.X)
            tot = spool.tile([128, NG], F32)
            nc.vector.reduce_sum(tot[:], sums[:], axis=mybir.AxisListType.X)

            # --- rearrange sums [128, g] -> pooledT [c, b], b = g*2 + h ---
            pooledT = spool.tile([C, B_LOC], F32)
            for g in range(NG):
                for h in range(NB_PER_G):
                    b_i = g * NB_PER_G + h
                    nc.sync.dma_start(pooledT[:, b_i:b_i + 1],
                                      tot[h * C:(h + 1) * C, g:g + 1])

            # --- excite MLP ---
            # hiddenT[m, b] = relu((w_down @ pooled)/65536 + b_down)
            ph = ppool.tile([C_MID, B_LOC], F32)
            nc.tensor.matmul(ph[:], wdT[:], pooledT[:])
            hT = spool.tile([C_MID, B_LOC], F32)
            nc.scalar.activation(hT[:], ph[:], mybir.ActivationFunctionType.Relu,
                                 bias=bdT[:], scale=1.0 / float(SPATIAL))
            # scaleT[c, b] = sigmoid(w_up @ hidden + b_up)
            ps = ppool.tile([C, B_LOC], F32)
            nc.tensor.matmul(ps[:], wuT[:], hT[:])
            sc = spool.tile([C, B_LOC], F32)
            nc.scalar.activation(sc[:], ps[:], mybir.ActivationFunctionType.Sigmoid,
                                 bias=buT[:], scale=1.0)

            # --- rearrange scaleT [c, b] -> scl [128, g] row layout ---
            scl = spool.tile([128, NG], F32)
            for g in range(NG):
                for h in range(NB_PER_G):
                    b_i = g * NB_PER_G + h
                    nc.sync.dma_start(scl[h * C:(h + 1) * C, g:g + 1],
                                      sc[:, b_i:b_i + 1])

            # --- pass 2: y = x * scale[row] ---
            for g in range(NG):
                for s in range(NS):
                    tin = io_pool.tile([128, T], F32, tag="io")
                    nc.gpsimd.dma_start(tin[:], x_t[g, :, s, :])
                    nc.vector.tensor_scalar_mul(tin[:], tin[:], scl[:, g:g + 1])
                    nc.sync.dma_start(y_t[g, :, s, :], tin[:])

    nc.compile()
    _NC = nc
    return nc


def kernel(trans_b, w_down, b_down, w_up, b_up):
    global LAST_RESULT
    nc = _build()

    trans_b = np.ascontiguousarray(np.asarray(trans_b, dtype=np.float32))
    w_down = np.ascontiguousarray(np.asarray(w_down, dtype=np.float32))
    b_down = np.ascontiguousarray(np.asarray(b_down, dtype=np.float32))
    w_up = np.ascontiguousarray(np.asarray(w_up, dtype=np.float32))
    b_up = np.ascontiguousarray(np.asarray(b_up, dtype=np.float32))

    x_flat = trans_b.reshape(B, C * H * W).reshape(B * C, SPATIAL)
    in_maps = []
    for i in range(N_CORES):
        in_maps.append({
            "x": x_flat[i * ROWS:(i + 1) * ROWS],
            "w_down": w_down,
            "b_down": b_down,
            "w_up": w_up,
            "b_up": b_up,
        })

    res = run_bass_kernel_spmd(nc, in_maps, core_ids=list(range(N_CORES)),
                               trace=TRACE)
    LAST_RESULT = res

    out = np.concatenate([res.results[i]["y"] for i in range(N_CORES)], axis=0)
    return out.reshape(B, C, H, W)


# revision 11
# speedup vs baseline: 1.2057x; 1.2057x over previous
"""SE (squeeze-excite) block for x[32,64,256,256] f32 on 8 TRN2 NeuronCores.

Data-parallel over batch: 4 batches per core, SE weights replicated.
Per core: x viewed as [256 rows = (4b x 64c), 65536 spatial], split into
32 chunks of [128 partitions, 4096] (2 MiB DMAs).

  Pass 1: stream chunks, DVE reduce_sum -> row sums. The last N_CACHE
          chunks stay resident in SBUF and are not re-read in pass 2.
  MLP:    two tiny PE matmuls (64->4 relu, 4->64 sigmoid); per-partition
          biases on the scalar engine; the 1/65536 mean scale is folded
          into the relu activation's scale argument.
  Pass 2: cached chunks are scaled in place and stored; the rest are
          re-streamed, scaled, stored.

HBM traffic per core: (2R + 1W - cached) * 64 MiB -> memory-bound.
"""

import numpy as np

import concourse.bacc as bacc
import concourse.bass as bass
import concourse.mybir as mybir
from concourse import tile
from concourse.bass_utils import run_bass_kernel_spmd

N_CORES = 8
B, C, H, W = 32, 64, 256, 256
C_MID = 4
B_LOC = B // N_CORES            # 4 batches per core
ROWS = B_LOC * C                # 256 (b,c) rows per core
SPATIAL = H * W                 # 65536
NG = ROWS // 128                # 2 partition groups
NB_PER_G = 128 // C             # 2 batches per partition group
T = 4096                        # spatial chunk (16KB/partition, 2MiB/DMA)
NS = SPATIAL // T               # 16 chunks per group
N_CHUNKS = NG * NS              # 32 chunks total
N_CACHE = 7                     # chunks kept resident in SBUF
N_STREAM_BUFS = 3
F32 = mybir.dt.float32

TRACE = False
LAST_RESULT = None

_NC = None


def _chunk_order():
    """(g, s) pairs in pass-1 emission order: streamed first, cached last."""
    order = [(g, s) for g in range(NG) for s in range(NS)]
    return order[:N_CHUNKS - N_CACHE], order[N_CHUNKS - N_CACHE:]


def _build():
    global _NC
    if _NC is not None:
        return _NC

    nc = bacc.Bacc("TRN2", debug=False)

    x = nc.dram_tensor("x", [ROWS, SPATIAL], F32, kind="ExternalInput")
    wd = nc.dram_tensor("w_down", [C_MID, C], F32, kind="ExternalInput")
    bd = nc.dram_tensor("b_down", [C_MID], F32, kind="ExternalInput")
    wu = nc.dram_tensor("w_up", [C, C_MID], F32, kind="ExternalInput")
    bu = nc.dram_tensor("b_up", [C], F32, kind="ExternalInput")
    y = nc.dram_tensor("y", [ROWS, SPATIAL], F32, kind="ExternalOutput")

    x_t = x.ap().rearrange("(g p) (s t) -> g p s t", p=128, t=T)
    y_t = y.ap().rearrange("(g p) (s t) -> g p s t", p=128, t=T)

    streamed, cached = _chunk_order()

    with tile.TileContext(nc) as tc:
        with (
            tc.tile_pool(name="const", bufs=1) as cpool,
            tc.tile_pool(name="io", bufs=N_STREAM_BUFS) as io_pool,
            tc.tile_pool(name="cache", bufs=N_CACHE) as cache_pool,
            tc.tile_pool(name="stats", bufs=1) as spool,
            tc.tile_pool(name="psum", bufs=1, space=bass.MemorySpace.PSUM) as ppool,
        ):
            # --- packed constants: one SBUF page ---
            # cols 0:4   partitions 0:64  -> w_down^T  [c, m]
            # cols 4:68  partitions 0:4   -> w_up^T    [m, c]
            # col  68    partitions 0:4   -> b_down    [m, 1]
            # col  69    partitions 0:64  -> b_up      [c, 1]
            const_t = cpool.tile([128, 70], F32)
            wdT = const_t[0:C, 0:C_MID]
            wuT = const_t[0:C_MID, C_MID:C_MID + C]
            bdT = const_t[0:C_MID, 68:69]
            buT = const_t[0:C, 69:70]
            nc.sync.dma_start(wdT, wd.ap().rearrange("m c -> c m"))
            nc.sync.dma_start(wuT, wu.ap().rearrange("c m -> m c"))
            nc.sync.dma_start(bdT, bd.ap().unsqueeze(1))
            nc.sync.dma_start(buT, bu.ap().unsqueeze(1))

            # --- packed stats: one SBUF page ---
            # cols 0:32  -> per-chunk row sums [128, (g s)]
            # cols 34:38 partitions 0:64 -> pooledT [c, b]
            # cols 38:42 partitions 0:64 -> sc (sigmoid scale) [c, b]
            # cols 44:48 partitions 0:4  -> hT (relu hidden) [m, b]
            stats_t = spool.tile([128, 48], F32)
            sums = stats_t[:, 0:N_CHUNKS].rearrange("p (g s) -> p g s", g=NG)
            pooledT = stats_t[0:C, 34:38]
            sc = stats_t[0:C, 38:42]
            hT = stats_t[0:C_MID, 44:48]
            tot = spool.tile([128, NG], F32)
            scl = spool.tile([128, NG], F32)

            cache_tiles = {}

            # --- pass 1: row sums over spatial ---
            for g, s in streamed:
                tin = io_pool.tile([128, T], F32, tag="io")
                nc.gpsimd.dma_start(tin[:], x_t[g, :, s, :])
                nc.vector.reduce_sum(sums[:, g, s:s + 1], tin[:],
                                     axis=mybir.AxisListType.X)
            for g, s in cached:
                ct = cache_pool.tile([128, T], F32, tag="cache")
                cache_tiles[(g, s)] = ct
                nc.gpsimd.dma_start(ct[:], x_t[g, :, s, :])
                nc.vector.reduce_sum(sums[:, g, s:s + 1], ct[:],
                                     axis=mybir.AxisListType.X)
            nc.vector.reduce_sum(tot[:], sums[:], axis=mybir.AxisListType.X)

            # --- rearrange tot [128, g] -> pooledT [c, b], b = g*2 + h ---
            for g in range(NG):
                for h in range(NB_PER_G):
                    b_i = g * NB_PER_G + h
                    nc.sync.dma_start(pooledT[:, b_i:b_i + 1],
                                      tot[h * C:(h + 1) * C, g:g + 1])

            # --- excite MLP ---
            # hiddenT[m, b] = relu((w_down @ sum) / 65536 + b_down)
            ph = ppool.tile([C_MID, B_LOC], F32)
            nc.tensor.matmul(ph[:], wdT, pooledT)
            nc.scalar.activation(hT, ph[:], mybir.ActivationFunctionType.Relu,
                                 bias=bdT, scale=1.0 / float(SPATIAL))
            # scaleT[c, b] = sigmoid(w_up @ hidden + b_up)
            ps = ppool.tile([C, B_LOC], F32)
            nc.tensor.matmul(ps[:], wuT, hT)
            nc.scalar.activation(sc, ps[:], mybir.ActivationFunctionType.Sigmoid,
                                 bias=buT, scale=1.0)

            # --- rearrange sc [c, b] -> scl [128, g] row layout ---
            for g in range(NG):
                for h in range(NB_PER_G):
                    b_i = g * NB_PER_G + h
                    nc.sync.dma_start(scl[h * C:(h + 1) * C, g:g + 1],
                                      sc[:, b_i:b_i + 1])

            # --- pass 2: y = x * scale[row] ---
            # cached chunks first: ready as soon as scl is, no load needed
            for g, s in cached:
                ct = cache_tiles[(g, s)]
                nc.vector.tensor_scalar_mul(ct[:], ct[:], scl[:, g:g + 1])
                nc.sync.dma_start(y_t[g, :, s, :], ct[:])
            for g, s in streamed:
                tin = io_pool.tile([128, T], F32, tag="io")
                nc.gpsimd.dma_start(tin[:], x_t[g, :, s, :])
                nc.vector.tensor_scalar_mul(tin[:], tin[:], scl[:, g:g + 1])
                nc.sync.dma_start(y_t[g, :, s, :], tin[:])

    nc.compile()
    _NC = nc
    return nc


def kernel(trans_b, w_down, b_down, w_up, b_up):
    global LAST_RESULT
    nc = _build()

    trans_b = np.ascontiguousarray(np.asarray(trans_b, dtype=np.float32))
    w_down = np.ascontiguousarray(np.asarray(w_down, dtype=np.float32))
    b_down = np.ascontiguousarray(np.asarray(b_down, dtype=np.float32))
    w_up = np.ascontiguousarray(np.asarray(w_up, dtype=np.float32))
    b_up = np.ascontiguousarray(np.asarray(b_up, dtype=np.float32))

    x_flat = trans_b.reshape(B * C, SPATIAL)
    in_maps = []
    for i in range(N_CORES):
        in_maps.append({
            "x": x_flat[i * ROWS:(i + 1) * ROWS],
            "w_down": w_down,
            "b_down": b_down,
            "w_up": w_up,
            "b_up": b_up,
        })

    res = run_bass_kernel_spmd(nc, in_maps, core_ids=list(range(N_CORES)),
                               trace=TRACE)
    LAST_RESULT = res

    out = np.concatenate([res.results[i]["y"] for i in range(N_CORES)], axis=0)
    return out.reshape(B, C, H, W)
